# revision 1
# baseline (speedup 1.0000x reference)
"""Linformer-style multi-head attention on 8 Trainium2 NeuronCores.

Problem (hardcoded): B=4, S=4096, C=1024, H=16, D=64, DK=256, fp32 in/out.

Sharding: core i handles (batch b = i//2, head-group g = i%2 of 8 heads).
Each core computes its 8 heads' attention and the partial output
projection out_part = head_out_g @ Wo[:, g_cols].T; the host sums the two
head-group partials per batch and adds bo.

All matmul operands are bf16 (host-cast); PSUM accumulation is fp32.
rel-err budget is 2e-2; bf16 end-to-end lands ~7e-3.

Per-core kernel:
  pass 1 (x streamed once in 8 s-chunks of 512):
      K,V = x @ Wk^T, x @ Wv^T        (layout [s, hd], bf16 in SBUF)
      Kp[hd,dk]  += K-chunk vs E^T     (PSUM accumulators, full-seq sum)
      VpT[dk,hd] += F^T vs V-chunk     (PSUM accumulators)
      Q^T[hd,s] per chunk, kept resident; chunks 6-7's x stays resident
      in SBUF (prefetched) and their Q^T is deferred into early pass 2
      as PE filler for the software-pipeline rampup.
  pass 2, software-pipelined over (chunk, pair) items (scores+exp run
  DEPTH=4 items ahead of AV/normalize); chunk ch's output projection is
  spread one s-tile per item across chunk ch+1's items so the PE has
  independent work while the DVE drains the normalize chain:
      scoresT[dk,s] = Kp-slices x Q^T   (row-packed head pairs, K=64,
          concurrent via PE row groups at partition bases 0/64)
      expT = exp(scoresT/8) on ACT      (batched [128,1024] per head row)
      AV and the softmax denominators run as col-tiled M=64 matmul
          pairs (col bases 0/64 stream concurrently): av bank 0 =
          [AV_h0 | AV_h1] over partitions, bank 1 = [den_h0 | den_h1].
          ONE full-width reciprocal + ONE full-width mul normalize both
          heads (DVE), writing bf16 ho.
      out[s,c] = ho-slices x WoT        (dedicated 2-slot PSUM pool;
          PSUM->SBUF copies on DVE into 4 staging buffers; out-DMA
          issues all on sync, idle in pass 2 and the fastest DGE)
  PSUM budget (8 banks): scps 2x[128,2,512]=4, avps 1x[128,2,512]=2,
  outps 2x[128,512]=2.
"""

import threading

import numpy as np

B, S, C = 4, 4096, 1024
H, D, DK = 16, 64, 256
HG = 8               # heads per core
HD = HG * D          # 512
NCORES = 8
SCH = 512            # sequence chunk
NCH = S // SCH       # 8 chunks
NST = SCH // 128     # 4 s-tiles per chunk
NCT = C // 128       # 8 c-tiles
NPT = HD // 128      # 4 hd blocks (head pairs)
NDB = DK // 128      # 2 dk blocks

_lock = threading.Lock()
_compiled = None


def _build():
    import concourse.bacc as bacc
    import concourse.bass as bass
    import concourse.tile as tile
    from concourse import mybir

    F32 = mybir.dt.float32
    BF16 = mybir.dt.bfloat16
    EXP = mybir.ActivationFunctionType.Exp

    nc = bacc.Bacc(None, target_bir_lowering=False)

    xT = nc.dram_tensor("xt", [C, S], BF16, kind="ExternalInput")
    wqT = nc.dram_tensor("wqt", [C, HD], BF16, kind="ExternalInput")
    wkT = nc.dram_tensor("wkt", [C, HD], BF16, kind="ExternalInput")
    wvT = nc.dram_tensor("wvt", [C, HD], BF16, kind="ExternalInput")
    ewT = nc.dram_tensor("ewt", [S, DK], BF16, kind="ExternalInput")
    fwT = nc.dram_tensor("fwt", [S, DK], BF16, kind="ExternalInput")
    eb = nc.dram_tensor("eb", [DK], F32, kind="ExternalInput")
    fb = nc.dram_tensor("fb", [DK], F32, kind="ExternalInput")
    woT = nc.dram_tensor("wot", [HD, C], BF16, kind="ExternalInput")
    ones = nc.dram_tensor("ones", [128, 64], BF16, kind="ExternalInput")
    out = nc.dram_tensor("out", [S, C], F32, kind="ExternalOutput")

    xT_r = xT[:].rearrange("(ct p) s -> ct p s", p=128)     # [8,128,4096]
    wq_r = wqT[:].rearrange("(ct p) n -> ct p n", p=128)    # [8,128,512]
    wk_r = wkT[:].rearrange("(ct p) n -> ct p n", p=128)
    wv_r = wvT[:].rearrange("(ct p) n -> ct p n", p=128)
    ew_r = ewT[:].rearrange("(st p) k -> st p k", p=128)    # [32,128,256]
    fw_r = fwT[:].rearrange("(st p) k -> st p k", p=128)
    wo_r = woT[:].rearrange("(pt p) c -> pt p c", p=128)    # [4,128,1024]

    with tile.TileContext(nc) as tc:
        with (
            tc.tile_pool(name="consts", bufs=1) as consts,
            tc.tile_pool(name="mids", bufs=1) as mids,
        ):
            wq_sb = consts.tile([128, NCT, HD], BF16)
            wk_sb = consts.tile([128, NCT, HD], BF16)
            wv_sb = consts.tile([128, NCT, HD], BF16)
            eb_sb = consts.tile([128, DK], F32)
            fb_sb = consts.tile([128, NDB], F32)

            kp_sb = mids.tile([128, NPT, DK], BF16)     # Kp [hd, dk]
            # Vp^T per dk-tile: head h at cols h*64..h*64+64. AV runs as
            # col-tiled M=64 matmul pairs (col bases 0/64 stream
            # concurrently through separate XBUSes), so no ones
            # augmentation is needed; the denominators come from a
            # separate col-tiled pair with an all-ones stationary.
            vp_sb = mids.tile([128, NDB, HD], BF16)
            ones_sb = consts.tile([128, 64], BF16)
            qt_sb = mids.tile([128, NCH * NPT, SCH], BF16)  # Q^T, all chunks
            wo_sb = mids.tile([128, NPT, C], BF16)
            # chunk-6/7 x stays resident: their Q^T is deferred into early
            # pass 2 as PE filler for the software-pipeline rampup.
            xt6 = mids.tile([128, NCT, SCH], BF16)
            xt7 = mids.tile([128, NCT, SCH], BF16)

            # ---------------- pass 1: K/V/Q projections + Kp/VpT ----------
            with (
                tc.tile_pool(name="p1sbuf", bufs=2) as p1s,
                tc.tile_pool(name="p1kv", bufs=1) as p1kv,
                tc.tile_pool(name="p1psum", bufs=2, space="PSUM") as p1ps,
                tc.tile_pool(name="qtpsum", bufs=2, space="PSUM") as qtps,
                tc.tile_pool(name="accps", bufs=1, space="PSUM") as accps,
            ):
                kp_ps = accps.tile([128, NPT, DK], F32)
                vp_ps = accps.tile([128, NDB, HD], F32)
                # chunk-0 x first, split fine so the first matmul's
                # stationary tile (xt0[:, 0, 0:128]) lands fast, then the
                # weights in first-use order.
                xt0 = p1s.tile([128, NCT, SCH], BF16, name="xt1")
                # First-use tiles land in dependency order: the first
                # matmul (split into two N=256 halves below) needs only
                # xt0[:, 0, 0:128] + wk cols 0:256, so those two pieces
                # go first on separate queues.
                nc.sync.dma_start(xt0[:, 0, 0:128], xT_r[0, :, 0:128])
                for q in range(2):
                    nc.sync.dma_start(
                        wk_sb[:, 0, q * 256:(q + 1) * 256],
                        wk_r[0][:, q * 256:(q + 1) * 256],
                    )
                for q in range(1, 4):
                    nc.sync.dma_start(
                        xt0[:, 0, q * 128:(q + 1) * 128],
                        xT_r[0, :, q * 128:(q + 1) * 128],
                    )
                # spread startup DMA issue across the three DMA-capable
                # engines — ~0.6us of issue per descriptor serializes per
                # engine, and sync alone would take ~24us to issue the
                # chunk-0 working set.
                for ct in range(1, NCT):
                    nc.sync.dma_start(xt0[:, ct, :], xT_r[ct, :, 0:SCH])
                    nc.sync.dma_start(wk_sb[:, ct, :], wk_r[ct])
                for ct in range(NCT):
                    nc.sync.dma_start(wv_sb[:, ct, :], wv_r[ct])
                ew0 = p1s.tile([128, NST, DK], BF16, name="ew")
                fw0 = p1s.tile([128, NST, DK], BF16, name="fw")
                for st in range(NST):
                    nc.sync.dma_start(ew0[:, st, :], ew_r[st])
                    nc.sync.dma_start(fw0[:, st, :], fw_r[st])
                for ct in range(NCT):
                    nc.sync.dma_start(wq_sb[:, ct, :], wq_r[ct])
                eb_bc = bass.AP(tensor=eb[:].tensor, offset=0, ap=[[0, 128], [1, DK]])
                nc.sync.dma_start(eb_sb[:], eb_bc)
                for db in range(NDB):
                    fb_col = fb[db * 128:(db + 1) * 128].rearrange(
                        "(p one) -> p one", one=1
                    )
                    nc.sync.dma_start(fb_sb[:, db:db + 1], fb_col)
                for pt in range(NPT):
                    nc.sync.dma_start(wo_sb[:, pt, :], wo_r[pt])
                nc.sync.dma_start(ones_sb, ones[:])

                for ch in range(NCH):
                    if ch == 0:
                        xt, ew, fw = xt0, ew0, fw0
                    else:
                        if ch < NCH - 2:
                            xt = p1s.tile([128, NCT, SCH], BF16, name="xt1")
                            for ct in range(NCT):
                                nc.sync.dma_start(
                                    xt[:, ct, :],
                                    xT_r[ct, :, ch * SCH:(ch + 1) * SCH],
                                )
                        else:
                            xt = xt6 if ch == NCH - 2 else xt7  # prefetched
                        ew = p1s.tile([128, NST, DK], BF16, name="ew")
                        fw = p1s.tile([128, NST, DK], BF16, name="fw")
                        for st in range(NST):
                            nc.sync.dma_start(ew[:, st, :], ew_r[ch * NST + st])
                            nc.sync.dma_start(fw[:, st, :], fw_r[ch * NST + st])
                    if ch == NCH - 3:
                        for ct in range(NCT):
                            nc.sync.dma_start(
                                xt6[:, ct, :],
                                xT_r[ct, :, (NCH - 2) * SCH:(NCH - 1) * SCH],
                            )
                    if ch == NCH - 2:
                        for ct in range(NCT):
                            nc.sync.dma_start(
                                xt7[:, ct, :],
                                xT_r[ct, :, (NCH - 1) * SCH:NCH * SCH],
                            )
                    k_sb = p1kv.tile([128, NST, HD], BF16, name="k_sb")
                    v_sb = p1kv.tile([128, NST, HD], BF16, name="v_sb")
                    # chunk 0 runs all K s-tiles before any V so the PE
                    # (in-order) doesn't stall on wv slices still in
                    # flight during the DMA rampup.
                    if ch == 0:
                        kv_order = [(0, st) for st in range(NST)] + \
                                   [(1, st) for st in range(NST)]
                    else:
                        kv_order = [(kv, st) for st in range(NST)
                                    for kv in range(2)]
                    for kv, st in kv_order:
                        w_sb = wk_sb if kv == 0 else wv_sb
                        dst = k_sb if kv == 0 else v_sb
                        ps = p1ps.tile([128, HD], F32, name="kvps")
                        for ct in range(NCT):
                            nc.tensor.matmul(
                                ps,
                                xt[:, ct, st * 128:(st + 1) * 128],
                                w_sb[:, ct, :],
                                start=(ct == 0), stop=(ct == NCT - 1),
                            )
                        nc.vector.tensor_copy(dst[:, st, :], ps)
                    first = ch == 0
                    last = ch == NCH - 1
                    for st in range(NST):
                        for pt in range(NPT):
                            # kp_ps slices pt={0,1} share PSUM bank 0 and
                            # pt={2,3} share bank 1 — one accumulation group
                            # per bank: start on the bank's first slice,
                            # stop on its last.
                            nc.tensor.matmul(
                                kp_ps[:, pt, :],
                                k_sb[:, st, pt * 128:(pt + 1) * 128],
                                ew[:, st, :],
                                start=(first and st == 0 and pt % 2 == 0),
                                stop=(last and st == NST - 1 and pt % 2 == 1),
                            )
                        for db in range(NDB):
                            nc.tensor.matmul(
                                vp_ps[:, db, :],
                                fw[:, st, db * 128:(db + 1) * 128],
                                v_sb[:, st, :],
                                start=(first and st == 0),
                                stop=(last and st == NST - 1),
                            )
                    # Q^T for the last two chunks is deferred into early
                    # pass 2, interleaved between attention items:
                    # relocated PE work that fills the software-pipeline
                    # rampup and keeps HAM warm across the transition.
                    for pt in range(NPT if ch < NCH - 2 else 0):
                        qps = qtps.tile([128, SCH], F32, name="qps")
                        for ct in range(NCT):
                            nc.tensor.matmul(
                                qps,
                                wq_sb[:, ct, pt * 128:(pt + 1) * 128],
                                xt[:, ct, :],
                                start=(ct == 0), stop=(ct == NCT - 1),
                            )
                        nc.vector.tensor_copy(qt_sb[:, ch * NPT + pt, :], qps)
                for pt in range(NPT):
                    nc.vector.tensor_add(kp_sb[:, pt, :], kp_ps[:, pt, :], eb_sb)
                for db in range(NDB):
                    nc.vector.tensor_scalar_add(
                        vp_sb[:, db, :].rearrange("p (h d) -> p h d", d=64),
                        vp_ps[:, db, :].rearrange("p (h d) -> p h d", d=64),
                        fb_sb[:, db:db + 1],
                    )

            # ---------------- pass 2: attention + output projection -------
            # Software pipeline over (chunk, pair) items: scores+exp
            # (stage A) runs 3 items ahead of AV/normalize (stage B).
            # Chunk ch's output projection is spread one s-tile per item
            # across chunk ch+1's items, so the PE always has independent
            # outproj work while the DVE drains the normalize chain.
            # PSUM budget (8 banks): scps 2x[128,2,512] = 4, avps
            # 1x[128,2,512] = 2, outps 2x[128,512] = 2.
            with (
                tc.tile_pool(name="p2ex", bufs=5) as p2ex,
                tc.tile_pool(name="p2ho", bufs=3) as p2ho,
                tc.tile_pool(name="p2rc", bufs=1) as p2rc,
                tc.tile_pool(name="p2out", bufs=4) as p2out,
                tc.tile_pool(name="scps", bufs=2, space="PSUM") as scps,
                tc.tile_pool(name="avps", bufs=1, space="PSUM") as avps,
                tc.tile_pool(name="outps", bufs=2, space="PSUM") as outps,
            ):
                ho_tiles = {}

                def stage_a(ch, pt):
                    qt_c = qt_sb[:, ch * NPT + pt, :]
                    ex = p2ex.tile([128, 2, NDB, SCH], BF16, name="ex")
                    for hrow in range(2):
                        lo, hi = hrow * 64, (hrow + 1) * 64
                        scp = scps.tile([128, NDB, SCH], F32, name="scp")
                        for j in range(NDB):
                            nc.tensor.matmul(
                                scp[:, j, :],
                                kp_sb[lo:hi, pt, j * 128:(j + 1) * 128],
                                qt_c[lo:hi, :],
                                start=True, stop=True,
                            )
                        nc.scalar.activation(
                            ex[:, hrow, :, :], scp, EXP, scale=0.125
                        )
                    return ex

                def stage_b(ch, pt, ex):
                    # per head-pair: bank 0 of av = [AV0 rows 0-63 | AV1
                    # rows 64-127], bank 1 = [den0 | den1], built from
                    # col-tiled M=64 matmul pairs (col bases 0/64 run
                    # concurrently). One full-width reciprocal and one
                    # full-width mul then normalize both heads at once.
                    if pt == 0:
                        ho_tiles[ch] = p2ho.tile(
                            [128, NPT, SCH], BF16, name="ho_sb"
                        )
                    ho_sb = ho_tiles[ch]
                    av = avps.tile([128, 2, SCH], F32, name="av")
                    for kt in range(NDB):
                        st_ = (kt == 0)
                        sp_ = (kt == NDB - 1)
                        for hrow in range(2):
                            h0 = (2 * pt + hrow) * 64
                            lo = hrow * 64
                            nc.tensor.matmul(
                                av[lo:lo + 64, 0, :],
                                vp_sb[:, kt, h0:h0 + 64],
                                ex[:, hrow, kt, :],
                                start=st_, stop=sp_,
                                skip_group_check=True,
                            )
                        for hrow in range(2):
                            lo = hrow * 64
                            nc.tensor.matmul(
                                av[lo:lo + 64, 1, :],
                                ones_sb,
                                ex[:, hrow, kt, :],
                                start=st_, stop=sp_,
                                skip_group_check=True,
                            )
                    rc = p2rc.tile([128, SCH], F32, name="rc")
                    nc.vector.reciprocal_approx_fast(rc, av[:, 1, :])
                    nc.vector.tensor_mul(ho_sb[:, pt, :], av[:, 0, :], rc)

                def outproj_st(ch, st):
                    ho_sb = ho_tiles[ch]
                    osb = p2out.tile([128, C], F32, name="osb")
                    row = ch * SCH + st * 128
                    for cc in range(2):
                        ops = outps.tile([128, 512], F32, name="ops")
                        for pt in range(NPT):
                            nc.tensor.matmul(
                                ops,
                                ho_sb[:, pt, st * 128:(st + 1) * 128],
                                wo_sb[:, pt, cc * 512:(cc + 1) * 512],
                                start=(pt == 0), stop=(pt == NPT - 1),
                            )
                        nc.vector.tensor_copy(osb[:, cc * 512:(cc + 1) * 512], ops)
                        # sync has no other pass-2 work and its HW DGE
                        # issues in ~0.6us vs gpsimd's ~1us software DGE,
                        # so all out-DMAs issue from sync.
                        eng = nc.sync
                        if ch == NCH - 1 and st == NST - 1:
                            # final tile: row halves on two queues to
                            # halve the end-of-kernel drain
                            for rh in range(2):
                                eng.dma_start(
                                    out[row + rh * 64:row + (rh + 1) * 64,
                                        cc * 512:(cc + 1) * 512],
                                    osb[rh * 64:(rh + 1) * 64,
                                        cc * 512:(cc + 1) * 512],
                                )
                        else:
                            eng.dma_start(
                                out[row:row + 128, cc * 512:(cc + 1) * 512],
                                osb[:, cc * 512:(cc + 1) * 512],
                            )
                    if st == NST - 1:
                        ho_tiles.pop(ch)

                def deferred_qt(qch, qpt):
                    xsrc = xt6 if qch == NCH - 2 else xt7
                    qps = outps.tile([128, 512], F32, name="ops")
                    for ct in range(NCT):
                        nc.tensor.matmul(
                            qps,
                            wq_sb[:, ct, qpt * 128:(qpt + 1) * 128],
                            xsrc[:, ct, :],
                            start=(ct == 0), stop=(ct == NCT - 1),
                        )
                    nc.vector.tensor_copy(qt_sb[:, qch * NPT + qpt, :], qps)

                items = [(ch, pt) for ch in range(NCH) for pt in range(NPT)]
                DEPTH = 4
                ex_tiles = {}
                # chunk 6/7's Q^T quarters: three interleaved with the
                # scores prologue (PE filler while DVE finalizes Kp/vpa),
                # the rest spread across the rampup items, which have no
                # output-projection filler yet.
                qlist = [(c, p) for c in (NCH - 2, NCH - 1) for p in range(NPT)]
                deferred_qt(*qlist[0])
                ex_tiles[items[0]] = stage_a(*items[0])
                deferred_qt(*qlist[1])
                for i in range(1, DEPTH):
                    ex_tiles[items[i]] = stage_a(*items[i])
                    if i == 1:
                        deferred_qt(*qlist[2])
                for i, (ch, pt) in enumerate(items):
                    if i < 5:
                        deferred_qt(*qlist[3 + i])
                    if i + DEPTH < len(items):
                        ex_tiles[items[i + DEPTH]] = stage_a(*items[i + DEPTH])
                    # outproj (independent PE work) is queued BEFORE
                    # stage_b: the PE runs its queue in order. The spread
                    # is shifted one item late so an item never runs an
                    # outproj whose ho normalize finished only one item
                    # ago: item (ch,0) runs the two-chunks-old final
                    # s-tile instead (needs p2ho bufs=3).
                    if pt == 0:
                        if ch >= 2:
                            outproj_st(ch - 2, NST - 1)
                    elif ch >= 1:
                        outproj_st(ch - 1, pt - 1)
                    stage_b(ch, pt, ex_tiles.pop((ch, pt)))
                outproj_st(NCH - 2, NST - 1)
                for st in range(NST):
                    outproj_st(NCH - 1, st)

    nc.compile()
    return nc


def get_compiled():
    global _compiled
    with _lock:
        if _compiled is None:
            _compiled = _build()
    return _compiled


def make_in_maps(x, Wq, Wk, Wv, E_w, E_b, F_w, F_b, Wo, bo):
    """Host-side sharding: core i -> (batch i//2, head-group i%2)."""
    import ml_dtypes

    f = np.float32
    bf = ml_dtypes.bfloat16
    x = np.asarray(x, f)
    ewT = np.ascontiguousarray(np.asarray(E_w, f).T).astype(bf)   # [S, DK]
    fwT = np.ascontiguousarray(np.asarray(F_w, f).T).astype(bf)
    in_maps = []
    for core in range(NCORES):
        b, g = divmod(core, 2)
        hs = slice(g * HG, (g + 1) * HG)
        wq = np.asarray(Wq, f)[hs].reshape(HD, C)
        wk = np.asarray(Wk, f)[hs].reshape(HD, C)
        wv = np.asarray(Wv, f)[hs].reshape(HD, C)
        wo = np.asarray(Wo, f)[:, g * HD:(g + 1) * HD]      # [C, 512]
        in_maps.append({
            "xt": np.ascontiguousarray(x[b].T).astype(bf),  # [C, S]
            "wqt": np.ascontiguousarray(wq.T).astype(bf),   # [C, HD]
            "wkt": np.ascontiguousarray(wk.T).astype(bf),
            "wvt": np.ascontiguousarray(wv.T).astype(bf),
            "ewt": ewT,
            "fwt": fwT,
            "eb": np.asarray(E_b, f),
            "fb": np.asarray(F_b, f),
            "wot": np.ascontiguousarray(wo.T).astype(bf),   # [HD, C]
            "ones": np.ones((128, 64), bf),
        })
    return in_maps


def assemble(results, bo):
    out = np.empty((B, S, C), np.float32)
    for b in range(B):
        out[b] = results[2 * b]["out"] + results[2 * b + 1]["out"]
    out += np.asarray(bo, np.float32)[None, None, :]
    return out


def kernel(x, Wq, Wk, Wv, E_w, E_b, F_w, F_b, Wo, bo):
    from concourse.bass_utils import run_bass_kernel_spmd

    nc = get_compiled()
    in_maps = make_in_maps(x, Wq, Wk, Wv, E_w, E_b, F_w, F_b, Wo, bo)
    res = run_bass_kernel_spmd(nc, in_maps, core_ids=list(range(NCORES)))
    return assemble(res.results, bo)



# revision 6
# speedup vs baseline: 1.1456x; 1.1456x over previous
"""Linformer-style multi-head attention on 8 Trainium2 NeuronCores.

Problem (hardcoded): B=4, S=4096, C=1024, H=16, D=64, DK=256, fp32 in/out.

Sharding: core i handles (batch b = i//2, head-group g = i%2 of 8 heads).
Each core computes its 8 heads' attention and the partial output
projection out_part = head_out_g @ Wo[:, g_cols].T; the host sums the two
head-group partials per batch and adds bo.

All matmul operands are bf16 (host-cast); PSUM accumulation is fp32.
rel-err budget is 2e-2; bf16 end-to-end lands ~7e-3.

Key algebraic reordering vs the obvious schedule: K and V are NEVER
materialized. The Linformer projections commute with the head
projections:
    Kp = Wk @ (x^T E^T),  Vp^T = (x^T F^T)^T @ Wv^T
so we first compute xEF = x^T @ [E^T | F^T]  ([C, 2*DK], contraction
over the full sequence), then the tiny [C]-contractions produce
Kp [hd, dk] and Vp^T [dk, hd]. This removes the x @ Wk / x @ Wv GEMMs
(2 x 131k PE cycles) and replaces K/V projection+sequence-reduction
(328k cycles) with 131k + 16k.

Per-core phases (PE cycle counts at 2.4 GHz):
  A  xEF = x^T [E^T|F^T]: stream x [s,c] layout, accumulate 8 c-tile
     PSUM banks over all 32 s-tiles (131k cycles). Drains interleave
     into the last s-group (vector+gpsimd split) so phase B starts on
     a freed bank without a bubble.
  B  Q^T = Wq x^T: stream xT [c,s] layout per 512-chunk (131k).
     C (Kp/Vp, 16k) is issued between B's chunk 6 and chunk 7 so C's
     DVE bias-adds overlap B's tail and kp/vp are ready when D starts.
  D  attention + output projection, software-pipelined over (chunk,
     head-pair) items exactly as tuned previously: scoresT via
     row-group-packed K=64 matmul pairs, exp on ACT, AV+softmax-den
     via col-tiled M=64 matmul pairs, one full-width reciprocal+mul
     on DVE, outproj spread one s-tile per item one chunk late.
  PSUM: A holds all 8 banks; B+C hold 3 (qt) + 2 (kp) + 2 (vp);
  D holds scps 4 + avps 2 + outps 2.
"""

import threading

import numpy as np

B, S, C = 4, 4096, 1024
H, D, DK = 16, 64, 256
HG = 8               # heads per core
HD = HG * D          # 512
NCORES = 8
EF = 2 * DK          # stacked E^T|F^T columns: 512
SCH = 512            # sequence chunk (phase B / D)
NCH = S // SCH       # 8 chunks
NST = SCH // 128     # 4 s-tiles per chunk
NCT = C // 128       # 8 c-tiles
NPT = HD // 128      # 4 hd blocks (head pairs)
NDB = DK // 128      # 2 dk blocks
NSG = 8              # phase-A s-groups (512 rows each, 4-row interleave)

_lock = threading.Lock()
_compiled = None


def _build():
    import concourse.bacc as bacc
    import concourse.bass as bass
    import concourse.tile as tile
    from concourse import mybir

    F32 = mybir.dt.float32
    BF16 = mybir.dt.bfloat16
    EXP = mybir.ActivationFunctionType.Exp

    nc = bacc.Bacc(None, target_bir_lowering=False)

    xs = nc.dram_tensor("xs", [S, C], BF16, kind="ExternalInput")
    xT = nc.dram_tensor("xt", [C, S], BF16, kind="ExternalInput")
    efT = nc.dram_tensor("eft", [S, EF], BF16, kind="ExternalInput")
    wqT = nc.dram_tensor("wqt", [C, HD], BF16, kind="ExternalInput")
    wkT = nc.dram_tensor("wkt", [C, HD], BF16, kind="ExternalInput")
    wvT = nc.dram_tensor("wvt", [C, HD], BF16, kind="ExternalInput")
    eb = nc.dram_tensor("eb", [DK], F32, kind="ExternalInput")
    fb = nc.dram_tensor("fb", [DK], F32, kind="ExternalInput")
    woT = nc.dram_tensor("wot", [HD, C], BF16, kind="ExternalInput")
    ones = nc.dram_tensor("ones", [128, 64], BF16, kind="ExternalInput")
    out = nc.dram_tensor("out", [S, C], F32, kind="ExternalOutput")

    # phase-A x/ef tiles: partition p carries 4 consecutive s-rows
    # (8KB contiguous DRAM per partition -> one fat descriptor per
    # 512-row group). s-permutation is irrelevant: phase A only ever
    # CONTRACTS over s, with x and ef sharing the same permutation.
    xs_r = xs[:].rearrange("(g p four) c -> g p four c", p=128, four=4)
    ef_r = efT[:].rearrange("(g p four) k -> g p four k", p=128, four=4)
    xT_r = xT[:].rearrange("(ct p) s -> ct p s", p=128)     # [8,128,4096]
    wq_r = wqT[:].rearrange("(ct p) n -> ct p n", p=128)    # [8,128,512]
    wk_r = wkT[:].rearrange("(ct p) n -> ct p n", p=128)
    wv_r = wvT[:].rearrange("(ct p) n -> ct p n", p=128)
    wo_r = woT[:].rearrange("(pt p) c -> pt p c", p=128)    # [4,128,1024]

    with tile.TileContext(nc) as tc:
        with (
            tc.tile_pool(name="consts", bufs=1) as consts,
            tc.tile_pool(name="mids", bufs=1) as mids,
            tc.tile_pool(name="pbx", bufs=3) as pbx,
        ):
            wq_sb = consts.tile([128, NCT, HD], BF16)
            wk_sb = consts.tile([128, NCT, HD], BF16)
            wv_sb = consts.tile([128, NCT, HD], BF16)
            eb_sb = consts.tile([128, DK], F32)
            fb_sb = consts.tile([128, NDB], F32)
            ones_sb = consts.tile([128, 64], BF16)

            xef_sb = mids.tile([128, NCT, EF], BF16)    # [c, 2dk]
            kp_sb = mids.tile([128, NPT, DK], BF16)     # Kp [hd, dk]
            vp_sb = mids.tile([128, NDB, HD], BF16)     # Vp^T [dk, hd]
            qt_sb = mids.tile([128, NCH * NPT, SCH], BF16)  # Q^T, all chunks
            wo_sb = mids.tile([128, NPT, C], BF16)

            # ---------------- phase A: xEF = x^T [E^T|F^T] ----------------
            with (
                tc.tile_pool(name="pax", bufs=3) as pax,
                tc.tile_pool(name="paef", bufs=3) as paef,
                tc.tile_pool(name="accps", bufs=1, space="PSUM") as accps,
            ):
                xef_ps = accps.tile([128, NCT, EF], F32)    # 8 banks

                # group 0 split fine so the first matmul's operands land
                # fast; ef slice first (it is the first matmul's rhs).
                ef0 = paef.tile([128, 4, EF], BF16, name="ef")
                x0 = pax.tile([128, 4, C], BF16, name="xsg")
                nc.sync.dma_start(ef0[:, 0, :], ef_r[0][:, 0, :])
                nc.sync.dma_start(x0[:, 0, :], xs_r[0][:, 0, :])
                for f in range(1, 4):
                    nc.sync.dma_start(ef0[:, f, :], ef_r[0][:, f, :])
                    nc.sync.dma_start(x0[:, f, :], xs_r[0][:, f, :])
                # consts on scalar (idle until D): first-use order.
                for ct in range(NCT):
                    nc.scalar.dma_start(wq_sb[:, ct, :], wq_r[ct])
                for ct in range(NCT):
                    nc.scalar.dma_start(wk_sb[:, ct, :], wk_r[ct])
                    nc.scalar.dma_start(wv_sb[:, ct, :], wv_r[ct])
                eb_bc = bass.AP(tensor=eb[:].tensor, offset=0, ap=[[0, 128], [1, DK]])
                nc.scalar.dma_start(eb_sb[:], eb_bc)
                for db in range(NDB):
                    fb_col = fb[db * 128:(db + 1) * 128].rearrange(
                        "(p one) -> p one", one=1
                    )
                    nc.scalar.dma_start(fb_sb[:, db:db + 1], fb_col)
                nc.scalar.dma_start(ones_sb, ones[:])
                for pt in range(NPT):
                    nc.scalar.dma_start(wo_sb[:, pt, :], wo_r[pt])

                # B prefetch: xT chunk-pairs on gpsimd during A.
                xt_tiles = []
                for cp in range(2):
                    xt_t = pbx.tile([128, NCT, 2 * SCH], BF16, name="xtc")
                    for ct in range(NCT):
                        nc.gpsimd.dma_start(
                            xt_t[:, ct, :],
                            xT_r[ct, :, cp * 2 * SCH:(cp + 1) * 2 * SCH],
                        )
                    xt_tiles.append(xt_t)

                for g in range(NSG):
                    if g == 0:
                        x_t, ef_t = x0, ef0
                    else:
                        x_t = pax.tile([128, 4, C], BF16, name="xsg")
                        ef_t = paef.tile([128, 4, EF], BF16, name="ef")
                        nc.sync.dma_start(ef_t[:], ef_r[g])
                        nc.sync.dma_start(x_t[:], xs_r[g])
                    last_g = g == NSG - 1
                    for f in range(4):
                        first = g == 0 and f == 0
                        last = last_g and f == 3
                        for ct in range(NCT):
                            nc.tensor.matmul(
                                xef_ps[:, ct, :],
                                x_t[:, f, ct * 128:(ct + 1) * 128],
                                ef_t[:, f, :],
                                start=first, stop=last,
                            )
                            if last:
                                # interleaved drain: bank ct is final
                                # here; copy while PE continues ct+1..
                                # (gpsimd has no PSUM port: DVE/ACT split)
                                if ct % 2 == 0:
                                    nc.vector.tensor_copy(
                                        xef_sb[:, ct, :], xef_ps[:, ct, :]
                                    )
                                else:
                                    nc.scalar.copy(
                                        xef_sb[:, ct, :], xef_ps[:, ct, :]
                                    )

            # ---------------- phase B: Q^T, with C (Kp/Vp) at the tail ----
            with (
                tc.tile_pool(name="qtps", bufs=3, space="PSUM") as qtps,
                tc.tile_pool(name="kvps", bufs=1, space="PSUM") as kvps,
            ):
                kp_ps = kvps.tile([128, NPT, DK], F32)      # 2 banks
                vp_ps = kvps.tile([128, NDB, HD], F32)      # 2 banks

                def qt_chunk(ch, xt_t, sub):
                    for pt in range(NPT):
                        qps = qtps.tile([128, SCH], F32, name="qps")
                        for ct in range(NCT):
                            nc.tensor.matmul(
                                qps,
                                wq_sb[:, ct, pt * 128:(pt + 1) * 128],
                                xt_t[:, ct, sub * SCH:(sub + 1) * SCH],
                                start=(ct == 0), stop=(ct == NCT - 1),
                            )
                        nc.vector.tensor_copy(qt_sb[:, ch * NPT + pt, :], qps)

                def phase_c():
                    for pt in range(NPT):
                        for ct in range(NCT):
                            nc.tensor.matmul(
                                kp_ps[:, pt, :],
                                wk_sb[:, ct, pt * 128:(pt + 1) * 128],
                                xef_sb[:, ct, 0:DK],
                                start=(ct == 0 and pt % 2 == 0),
                                stop=(ct == NCT - 1 and pt % 2 == 1),
                            )
                    for db in range(NDB):
                        for ct in range(NCT):
                            nc.tensor.matmul(
                                vp_ps[:, db, :],
                                xef_sb[:, ct, DK + db * 128:DK + (db + 1) * 128],
                                wv_sb[:, ct, :],
                                start=(ct == 0), stop=(ct == NCT - 1),
                            )
                    for pt in range(NPT):
                        nc.vector.tensor_add(
                            kp_sb[:, pt, :], kp_ps[:, pt, :], eb_sb
                        )
                    for db in range(NDB):
                        # fb varies along the partition (dk) axis: ACT's
                        # per-partition bias-add fits, and keeps the DVE
                        # free for the qt copies.
                        nc.scalar.add(
                            vp_sb[:, db, :], vp_ps[:, db, :],
                            fb_sb[:, db:db + 1],
                        )

                for cp in range(4):
                    if cp < 2:
                        xt_t = xt_tiles[cp]             # prefetched in A
                    else:
                        xt_t = pbx.tile([128, NCT, 2 * SCH], BF16, name="xtc")
                        for ct in range(NCT):
                            nc.sync.dma_start(
                                xt_t[:, ct, :],
                                xT_r[ct, :, cp * 2 * SCH:(cp + 1) * 2 * SCH],
                            )
                    for sub in range(2):
                        ch = cp * 2 + sub
                        if ch == NCH - 1:
                            phase_c()   # C's DVE adds overlap B's tail
                        qt_chunk(ch, xt_t, sub)

            # ---------------- phase D: attention + output projection ------
            # Software pipeline over (chunk, pair) items: scores+exp
            # (stage A) runs DEPTH items ahead of AV/normalize (stage B).
            # Chunk ch's output projection is spread one s-tile per item
            # across chunk ch+1's items, so the PE always has independent
            # outproj work while the DVE drains the normalize chain.
            # PSUM budget (8 banks): scps 2x[128,2,512] = 4, avps
            # 1x[128,2,512] = 2, outps 2x[128,512] = 2.
            with (
                tc.tile_pool(name="p2ex", bufs=5) as p2ex,
                tc.tile_pool(name="p2ho", bufs=3) as p2ho,
                tc.tile_pool(name="p2rc", bufs=1) as p2rc,
                tc.tile_pool(name="p2out", bufs=4) as p2out,
                tc.tile_pool(name="scps", bufs=2, space="PSUM") as scps,
                tc.tile_pool(name="avps", bufs=1, space="PSUM") as avps,
                tc.tile_pool(name="outps", bufs=2, space="PSUM") as outps,
            ):
                ho_tiles = {}

                def stage_a(ch, pt):
                    qt_c = qt_sb[:, ch * NPT + pt, :]
                    ex = p2ex.tile([128, 2, NDB, SCH], BF16, name="ex")
                    for hrow in range(2):
                        lo, hi = hrow * 64, (hrow + 1) * 64
                        scp = scps.tile([128, NDB, SCH], F32, name="scp")
                        for j in range(NDB):
                            nc.tensor.matmul(
                                scp[:, j, :],
                                kp_sb[lo:hi, pt, j * 128:(j + 1) * 128],
                                qt_c[lo:hi, :],
                                start=True, stop=True,
                            )
                        nc.scalar.activation(
                            ex[:, hrow, :, :], scp, EXP, scale=0.125
                        )
                    return ex

                def stage_b(ch, pt, ex):
                    # per head-pair: bank 0 of av = [AV0 rows 0-63 | AV1
                    # rows 64-127], bank 1 = [den0 | den1], built from
                    # col-tiled M=64 matmul pairs (col bases 0/64 run
                    # concurrently). One full-width reciprocal and one
                    # full-width mul then normalize both heads at once.
                    if pt == 0:
                        ho_tiles[ch] = p2ho.tile(
                            [128, NPT, SCH], BF16, name="ho_sb"
                        )
                    ho_sb = ho_tiles[ch]
                    av = avps.tile([128, 2, SCH], F32, name="av")
                    for kt in range(NDB):
                        st_ = (kt == 0)
                        sp_ = (kt == NDB - 1)
                        for hrow in range(2):
                            h0 = (2 * pt + hrow) * 64
                            lo = hrow * 64
                            nc.tensor.matmul(
                                av[lo:lo + 64, 0, :],
                                vp_sb[:, kt, h0:h0 + 64],
                                ex[:, hrow, kt, :],
                                start=st_, stop=sp_,
                                skip_group_check=True,
                            )
                        for hrow in range(2):
                            lo = hrow * 64
                            nc.tensor.matmul(
                                av[lo:lo + 64, 1, :],
                                ones_sb,
                                ex[:, hrow, kt, :],
                                start=st_, stop=sp_,
                                skip_group_check=True,
                            )
                    rc = p2rc.tile([128, SCH], F32, name="rc")
                    nc.vector.reciprocal_approx_fast(rc, av[:, 1, :])
                    nc.vector.tensor_mul(ho_sb[:, pt, :], av[:, 0, :], rc)

                def outproj_st(ch, st):
                    ho_sb = ho_tiles[ch]
                    osb = p2out.tile([128, C], F32, name="osb")
                    row = ch * SCH + st * 128
                    for cc in range(2):
                        ops = outps.tile([128, 512], F32, name="ops")
                        for pt in range(NPT):
                            nc.tensor.matmul(
                                ops,
                                ho_sb[:, pt, st * 128:(st + 1) * 128],
                                wo_sb[:, pt, cc * 512:(cc + 1) * 512],
                                start=(pt == 0), stop=(pt == NPT - 1),
                            )
                        nc.vector.tensor_copy(osb[:, cc * 512:(cc + 1) * 512], ops)
                        # sync has no other phase-D work and its HW DGE
                        # issues in ~0.6us vs gpsimd's ~1us software DGE,
                        # so all out-DMAs issue from sync.
                        eng = nc.sync
                        if ch == NCH - 1 and st == NST - 1:
                            # final tile: row halves on two queues to
                            # halve the end-of-kernel drain
                            for rh in range(2):
                                eng.dma_start(
                                    out[row + rh * 64:row + (rh + 1) * 64,
                                        cc * 512:(cc + 1) * 512],
                                    osb[rh * 64:(rh + 1) * 64,
                                        cc * 512:(cc + 1) * 512],
                                )
                        else:
                            eng.dma_start(
                                out[row:row + 128, cc * 512:(cc + 1) * 512],
                                osb[:, cc * 512:(cc + 1) * 512],
                            )
                    if st == NST - 1:
                        ho_tiles.pop(ch)

                items = [(ch, pt) for ch in range(NCH) for pt in range(NPT)]
                DEPTH = 4
                ex_tiles = {}
                for i in range(DEPTH):
                    ex_tiles[items[i]] = stage_a(*items[i])
                for i, (ch, pt) in enumerate(items):
                    if i + DEPTH < len(items):
                        ex_tiles[items[i + DEPTH]] = stage_a(*items[i + DEPTH])
                    # outproj (independent PE work) is queued BEFORE
                    # stage_b: the PE runs its queue in order. The spread
                    # is shifted one item late so an item never runs an
                    # outproj whose ho normalize finished only one item
                    # ago: item (ch,0) runs the two-chunks-old final
                    # s-tile instead (needs p2ho bufs=3).
                    if pt == 0:
                        if ch >= 2:
                            outproj_st(ch - 2, NST - 1)
                    elif ch >= 1:
                        outproj_st(ch - 1, pt - 1)
                    stage_b(ch, pt, ex_tiles.pop((ch, pt)))
                outproj_st(NCH - 2, NST - 1)
                for st in range(NST):
                    outproj_st(NCH - 1, st)

    nc.compile()
    return nc


def get_compiled():
    global _compiled
    with _lock:
        if _compiled is None:
            _compiled = _build()
    return _compiled


def make_in_maps(x, Wq, Wk, Wv, E_w, E_b, F_w, F_b, Wo, bo):
    """Host-side sharding: core i -> (batch i//2, head-group i%2)."""
    import ml_dtypes

    f = np.float32
    bf = ml_dtypes.bfloat16
    x = np.asarray(x, f)
    efT = np.ascontiguousarray(
        np.concatenate([np.asarray(E_w, f).T, np.asarray(F_w, f).T], axis=1)
    ).astype(bf)                                        # [S, 2*DK]
    in_maps = []
    for core in range(NCORES):
        b, g = divmod(core, 2)
        hs = slice(g * HG, (g + 1) * HG)
        wq = np.asarray(Wq, f)[hs].reshape(HD, C)
        wk = np.asarray(Wk, f)[hs].reshape(HD, C)
        wv = np.asarray(Wv, f)[hs].reshape(HD, C)
        wo = np.asarray(Wo, f)[:, g * HD:(g + 1) * HD]      # [C, 512]
        in_maps.append({
            "xs": np.ascontiguousarray(x[b]).astype(bf),    # [S, C]
            "xt": np.ascontiguousarray(x[b].T).astype(bf),  # [C, S]
            "eft": efT,
            "wqt": np.ascontiguousarray(wq.T).astype(bf),   # [C, HD]
            "wkt": np.ascontiguousarray(wk.T).astype(bf),
            "wvt": np.ascontiguousarray(wv.T).astype(bf),
            "eb": np.asarray(E_b, f),
            "fb": np.asarray(F_b, f),
            "wot": np.ascontiguousarray(wo.T).astype(bf),   # [HD, C]
            "ones": np.ones((128, 64), bf),
        })
    return in_maps


def assemble(results, bo):
    out = np.empty((B, S, C), np.float32)
    for b in range(B):
        out[b] = results[2 * b]["out"] + results[2 * b + 1]["out"]
    out += np.asarray(bo, np.float32)[None, None, :]
    return out


def kernel(x, Wq, Wk, Wv, E_w, E_b, F_w, F_b, Wo, bo):
    from concourse.bass_utils import run_bass_kernel_spmd

    nc = get_compiled()
    in_maps = make_in_maps(x, Wq, Wk, Wv, E_w, E_b, F_w, F_b, Wo, bo)
    res = run_bass_kernel_spmd(nc, in_maps, core_ids=list(range(NCORES)))
    return assemble(res.results, bo)


# revision 9
# speedup vs baseline: 1.1809x; 1.0308x over previous
"""Linformer-style multi-head attention on 8 Trainium2 NeuronCores.

Problem (hardcoded): B=4, S=4096, C=1024, H=16, D=64, DK=256, fp32 in/out.

Sharding: core i handles (batch b = i//2, head-group g = i%2 of 8 heads).
Each core computes its 8 heads' attention and the partial output
projection out_part = head_out_g @ Wo[:, g_cols].T (emitted bf16); the
host sums the two head-group partials per batch and adds bo.

Key algebraic reordering: K and V are NEVER materialized. The Linformer
projections commute with the head projections:
    Kp = Wk @ (x^T E^T),  Vp^T = (x^T F^T)^T @ Wv^T
so we first compute xEF = x^T @ [E^T | F^T]  ([C, 2*DK], contraction
over the full sequence), then tiny [C]-contractions produce Kp [hd, dk]
and Vp^T [dk, hd]. This replaces the x@Wk / x@Wv GEMMs + sequence
reductions (328k PE cycles) with 131k + 16k.

Phase schedule (single stream; PE cycle counts at 2.4 GHz):
  warmup  8 throwaway matmuls on a memzero'd tile so the HAM p-state
          ramps while the first input DMAs land (PE idle >~1us drops
          the tensor engine to K=4/8 for 10-27us stretches).
  A||B    A = xEF accumulation in two c-half passes (4 PSUM banks each,
          pass 2 reuses pass 1's banks after an interleaved drain);
          B = Q^T = Wq x^T per (chunk, head-pair).  A-units (3.4us) and
          B-units (1.7us) interleave ~1:2 so HBM demand stays ~250 GB/s
          instead of phase-serial 380/70.  ef stays SBUF-resident for
          pass 2; x streams once (column halves).
  C       Kp/Vp from xEF, slotted between the tail B-units so its
          DVE/ACT bias-adds finish before D's first scores matmul.
  D       attention + output projection, software-pipelined over
          (chunk, head-pair) items: scoresT via row-group-packed K=64
          matmul pairs, exp on ACT, AV+softmax-den via col-tiled M=64
          matmul pairs, one full-width reciprocal+mul on DVE, outproj
          spread one s-tile per item one chunk late.  osb copies split
          DVE/ACT; out partials written bf16.
  PSUM: warm 1 | A 4 + qt 3 | kvps 4 + qt 3 | D: scps 4 + avps 2 +
  outps 2.
"""

import threading

import numpy as np

B, S, C = 4, 4096, 1024
H, D, DK = 16, 64, 256
HG = 8               # heads per core
HD = HG * D          # 512
NCORES = 8
EF = 2 * DK          # stacked E^T|F^T columns: 512
SCH = 512            # sequence chunk (phase B / D)
NCH = S // SCH       # 8 chunks
NST = SCH // 128     # 4 s-tiles per chunk
NCT = C // 128       # 8 c-tiles
NPT = HD // 128      # 4 hd blocks (head pairs)
NDB = DK // 128      # 2 dk blocks
NSG = 8              # phase-A s-groups (512 rows each, 4-row interleave)
CH2 = C // 2         # phase-A column half

_lock = threading.Lock()
_compiled = None


def _build():
    import concourse.bacc as bacc
    import concourse.bass as bass
    import concourse.tile as tile
    from concourse import mybir

    F32 = mybir.dt.float32
    BF16 = mybir.dt.bfloat16
    EXP = mybir.ActivationFunctionType.Exp

    nc = bacc.Bacc(None, target_bir_lowering=False)

    xs = nc.dram_tensor("xs", [S, C], BF16, kind="ExternalInput")
    xT = nc.dram_tensor("xt", [C, S], BF16, kind="ExternalInput")
    efT = nc.dram_tensor("eft", [S, EF], BF16, kind="ExternalInput")
    wqT = nc.dram_tensor("wqt", [C, HD], BF16, kind="ExternalInput")
    wkT = nc.dram_tensor("wkt", [C, HD], BF16, kind="ExternalInput")
    wvT = nc.dram_tensor("wvt", [C, HD], BF16, kind="ExternalInput")
    eb = nc.dram_tensor("eb", [DK], F32, kind="ExternalInput")
    fb = nc.dram_tensor("fb", [DK], F32, kind="ExternalInput")
    woT = nc.dram_tensor("wot", [HD, C], BF16, kind="ExternalInput")
    ones = nc.dram_tensor("ones", [128, 64], BF16, kind="ExternalInput")
    out = nc.dram_tensor("out", [S, C], BF16, kind="ExternalOutput")

    # phase-A x/ef tiles: partition p carries 4 consecutive s-rows (fat
    # contiguous DRAM reads per partition).  The s-permutation is
    # irrelevant: phase A only ever CONTRACTS over s, with x and ef
    # sharing the same permutation.
    xs_r = xs[:].rearrange("(g p four) c -> g p four c", p=128, four=4)
    ef_r = efT[:].rearrange("(g p four) k -> g p four k", p=128, four=4)
    xT_r = xT[:].rearrange("(ct p) s -> ct p s", p=128)     # [8,128,4096]
    wq_r = wqT[:].rearrange("(ct p) n -> ct p n", p=128)    # [8,128,512]
    wk_r = wkT[:].rearrange("(ct p) n -> ct p n", p=128)
    wv_r = wvT[:].rearrange("(ct p) n -> ct p n", p=128)
    wo_r = woT[:].rearrange("(pt p) c -> pt p c", p=128)    # [4,128,1024]

    with tile.TileContext(nc) as tc:
        with (
            tc.tile_pool(name="consts", bufs=1) as consts,
            tc.tile_pool(name="mids", bufs=1) as mids,
            tc.tile_pool(name="pbx", bufs=2) as pbx,
        ):
            warm_sb = consts.tile([128, 512], BF16)
            wq_sb = consts.tile([128, NCT, HD], BF16)
            wk_sb = consts.tile([128, NCT, HD], BF16)
            wv_sb = consts.tile([128, NCT, HD], BF16)
            eb_sb = consts.tile([128, DK], F32)
            fb_sb = consts.tile([128, NDB], F32)
            ones_sb = consts.tile([128, 64], BF16)

            ef_sb = mids.tile([128, NSG, 4, EF], BF16)  # resident E^T|F^T
            xef_sb = mids.tile([128, NCT, EF], BF16)    # [c, 2dk]
            kp_sb = mids.tile([128, NPT, DK], BF16)     # Kp [hd, dk]
            vp_sb = mids.tile([128, NDB, HD], BF16)     # Vp^T [dk, hd]
            qt_sb = mids.tile([128, NCH * NPT, SCH], BF16)  # Q^T, all chunks
            wo_sb = mids.tile([128, NPT, C], BF16)

            with tc.tile_pool(name="qtps", bufs=3, space="PSUM") as qtps:
                # ---- warmup: PE busy while the first input DMAs land --
                with tc.tile_pool(name="warmps", bufs=1, space="PSUM") as wps:
                    warm_ps = wps.tile([128, 512], F32)
                    nc.scalar.memzero(warm_sb[:])
                    for _ in range(8):
                        nc.tensor.matmul(
                            warm_ps, warm_sb[:, 0:128], warm_sb,
                            start=True, stop=True,
                        )

                # ---- DMA kickoff ------------------------------------
                # sync: A stream (g0 split fine so the first matmul's
                # operands land fast; ef slice first = first rhs).
                for f in range(4):
                    nc.sync.dma_start(ef_sb[:, 0, f, :], ef_r[0][:, f, :])
                # scalar: consts in first-use order (wq before B unit 0).
                nc.scalar.dma_start(ones_sb, ones[:])
                for ct in range(NCT):
                    nc.scalar.dma_start(wq_sb[:, ct, :], wq_r[ct])
                # gpsimd: all four xT chunk-pairs; pbx bufs=2 makes the
                # later dma_starts self-pace on the pool WAR semaphore
                # (gpsimd has nothing else to do).
                xt_tiles = []
                for cp in range(4):
                    xt_t = pbx.tile([128, NCT, 2 * SCH], BF16, name="xtc")
                    for ct in range(NCT):
                        nc.gpsimd.dma_start(
                            xt_t[:, ct, :],
                            xT_r[ct, :, cp * 2 * SCH:(cp + 1) * 2 * SCH],
                        )
                    xt_tiles.append(xt_t)
                # scalar: remaining consts (C/D-phase first-use order).
                for ct in range(NCT):
                    nc.scalar.dma_start(wk_sb[:, ct, :], wk_r[ct])
                eb_bc = bass.AP(tensor=eb[:].tensor, offset=0, ap=[[0, 128], [1, DK]])
                nc.scalar.dma_start(eb_sb[:], eb_bc)
                for ct in range(NCT):
                    nc.scalar.dma_start(wv_sb[:, ct, :], wv_r[ct])
                for db in range(NDB):
                    fb_col = fb[db * 128:(db + 1) * 128].rearrange(
                        "(p one) -> p one", one=1
                    )
                    nc.scalar.dma_start(fb_sb[:, db:db + 1], fb_col)
                for pt in range(NPT):
                    nc.scalar.dma_start(wo_sb[:, pt, :], wo_r[pt])

                # ---- A||B interleaved units -------------------------
                a_tiles = {}

                def fetch_a(p, g):
                    if (p, g) in a_tiles:
                        return a_tiles[(p, g)]
                    t = pax.tile([128, 4, CH2], BF16, name="xsg")
                    if p == 0 and g == 0:
                        for f in range(4):
                            nc.sync.dma_start(
                                t[:, f, :], xs_r[0][:, f, 0:CH2]
                            )
                    else:
                        nc.sync.dma_start(
                            t[:], xs_r[g][:, :, p * CH2:(p + 1) * CH2]
                        )
                    a_tiles[(p, g)] = t
                    return t

                def a_unit(p, g, xef_ps):
                    x_t = fetch_a(p, g)
                    if p == 0 and g + 1 < NSG:
                        nc.sync.dma_start(ef_sb[:, g + 1, :, :], ef_r[g + 1])
                    if g + 1 < NSG:
                        fetch_a(p, g + 1)
                    elif p == 0:
                        fetch_a(1, 0)
                    last_g = g == NSG - 1
                    for f in range(4):
                        first = g == 0 and f == 0
                        last = last_g and f == 3
                        for ct in range(4):
                            nc.tensor.matmul(
                                xef_ps[:, ct, :],
                                x_t[:, f, ct * 128:(ct + 1) * 128],
                                ef_sb[:, g, f, :],
                                start=first, stop=last,
                            )
                            if last:
                                # interleaved drain: bank ct final here;
                                # copy while the PE continues ct+1..
                                dst = xef_sb[:, p * 4 + ct, :]
                                if ct % 2 == 0:
                                    nc.vector.tensor_copy(dst, xef_ps[:, ct, :])
                                else:
                                    nc.scalar.copy(dst, xef_ps[:, ct, :])
                    a_tiles.pop((p, g))

                def b_unit(ch, pt):
                    xt_t = xt_tiles[ch // 2]
                    sub = ch % 2
                    qps = qtps.tile([128, SCH], F32, name="qps")
                    for ct in range(NCT):
                        nc.tensor.matmul(
                            qps,
                            wq_sb[:, ct, pt * 128:(pt + 1) * 128],
                            xt_t[:, ct, sub * SCH:(sub + 1) * SCH],
                            start=(ct == 0), stop=(ct == NCT - 1),
                        )
                    nc.vector.tensor_copy(qt_sb[:, ch * NPT + pt, :], qps)

                b_units = [(ch, pt) for ch in range(NCH) for pt in range(NPT)]
                bi = 0
                with (
                    tc.tile_pool(name="pax", bufs=3) as pax,
                    tc.tile_pool(name="accA", bufs=1, space="PSUM") as accA,
                ):
                    for p in range(2):
                        xef_ps = accA.tile([128, 4, EF], F32, name="xefps")
                        for g in range(NSG):
                            a_unit(p, g, xef_ps)
                            # lead with two A-only units (cold DMA);
                            # then 2 B-units per A-unit.
                            if p == 0 and g < 2:
                                continue
                            b_unit(*b_units[bi])
                            b_unit(*b_units[bi + 1])
                            bi += 2

                # ---- C: Kp/Vp, slotted between tail B-units ---------
                with tc.tile_pool(name="kvps", bufs=1, space="PSUM") as kvps:
                    kp_ps = kvps.tile([128, NPT, DK], F32)      # 2 banks
                    vp_ps = kvps.tile([128, NDB, HD], F32)      # 2 banks
                    b_unit(*b_units[bi])
                    bi += 1
                    for pt in range(NPT):
                        for ct in range(NCT):
                            nc.tensor.matmul(
                                kp_ps[:, pt, :],
                                wk_sb[:, ct, pt * 128:(pt + 1) * 128],
                                xef_sb[:, ct, 0:DK],
                                start=(ct == 0 and pt % 2 == 0),
                                stop=(ct == NCT - 1 and pt % 2 == 1),
                            )
                    for pt in range(NPT):
                        nc.vector.tensor_add(
                            kp_sb[:, pt, :], kp_ps[:, pt, :], eb_sb
                        )
                    b_unit(*b_units[bi])
                    bi += 1
                    for db in range(NDB):
                        for ct in range(NCT):
                            nc.tensor.matmul(
                                vp_ps[:, db, :],
                                xef_sb[:, ct, DK + db * 128:DK + (db + 1) * 128],
                                wv_sb[:, ct, :],
                                start=(ct == 0), stop=(ct == NCT - 1),
                            )
                    for db in range(NDB):
                        # fb varies along the partition (dk) axis: ACT's
                        # per-partition bias-add fits, keeping the DVE
                        # free for the qt copies.
                        nc.scalar.add(
                            vp_sb[:, db, :], vp_ps[:, db, :],
                            fb_sb[:, db:db + 1],
                        )
                    while bi < len(b_units):
                        b_unit(*b_units[bi])
                        bi += 1

            # ---------------- phase D: attention + output projection ------
            # Software pipeline over (chunk, pair) items: scores+exp
            # (stage A) runs DEPTH items ahead of AV/normalize (stage B).
            # Chunk ch's output projection is spread one s-tile per item
            # across chunk ch+1's items, so the PE always has independent
            # outproj work while the DVE drains the normalize chain.
            # PSUM pool creation order matters: avps/outps are created
            # first so they land on the banks the qt rotation still
            # reads at the transition; scps lands on the accA/kvps banks
            # that freed many units earlier, so D's first scores matmul
            # has no WAR wait.
            with (
                tc.tile_pool(name="p2ex", bufs=5) as p2ex,
                tc.tile_pool(name="p2ho", bufs=3) as p2ho,
                tc.tile_pool(name="p2rc", bufs=1) as p2rc,
                tc.tile_pool(name="p2out", bufs=4) as p2out,
                tc.tile_pool(name="avps", bufs=1, space="PSUM") as avps,
                tc.tile_pool(name="outps", bufs=2, space="PSUM") as outps,
                tc.tile_pool(name="scps", bufs=2, space="PSUM") as scps,
            ):
                ho_tiles = {}

                def stage_a(ch, pt):
                    qt_c = qt_sb[:, ch * NPT + pt, :]
                    ex = p2ex.tile([128, 2, NDB, SCH], BF16, name="ex")
                    for hrow in range(2):
                        lo, hi = hrow * 64, (hrow + 1) * 64
                        scp = scps.tile([128, NDB, SCH], F32, name="scp")
                        for j in range(NDB):
                            nc.tensor.matmul(
                                scp[:, j, :],
                                kp_sb[lo:hi, pt, j * 128:(j + 1) * 128],
                                qt_c[lo:hi, :],
                                start=True, stop=True,
                            )
                        nc.scalar.activation(
                            ex[:, hrow, :, :], scp, EXP, scale=0.125
                        )
                    return ex

                def stage_b(ch, pt, ex):
                    # per head-pair: bank 0 of av = [AV0 rows 0-63 | AV1
                    # rows 64-127], bank 1 = [den0 | den1], built from
                    # col-tiled M=64 matmul pairs (col bases 0/64 run
                    # concurrently). One full-width reciprocal and one
                    # full-width mul then normalize both heads at once.
                    if pt == 0:
                        ho_tiles[ch] = p2ho.tile(
                            [128, NPT, SCH], BF16, name="ho_sb"
                        )
                    ho_sb = ho_tiles[ch]
                    av = avps.tile([128, 2, SCH], F32, name="av")
                    for kt in range(NDB):
                        st_ = (kt == 0)
                        sp_ = (kt == NDB - 1)
                        for hrow in range(2):
                            h0 = (2 * pt + hrow) * 64
                            lo = hrow * 64
                            nc.tensor.matmul(
                                av[lo:lo + 64, 0, :],
                                vp_sb[:, kt, h0:h0 + 64],
                                ex[:, hrow, kt, :],
                                start=st_, stop=sp_,
                                skip_group_check=True,
                            )
                        for hrow in range(2):
                            lo = hrow * 64
                            nc.tensor.matmul(
                                av[lo:lo + 64, 1, :],
                                ones_sb,
                                ex[:, hrow, kt, :],
                                start=st_, stop=sp_,
                                skip_group_check=True,
                            )
                    rc = p2rc.tile([128, SCH], F32, name="rc")
                    nc.vector.reciprocal_approx_fast(rc, av[:, 1, :])
                    nc.vector.tensor_mul(ho_sb[:, pt, :], av[:, 0, :], rc)

                def outproj_st(ch, st):
                    ho_sb = ho_tiles[ch]
                    osb = p2out.tile([128, C], BF16, name="osb")
                    row = ch * SCH + st * 128
                    for cc in range(2):
                        ops = outps.tile([128, 512], F32, name="ops")
                        for pt in range(NPT):
                            nc.tensor.matmul(
                                ops,
                                ho_sb[:, pt, st * 128:(st + 1) * 128],
                                wo_sb[:, pt, cc * 512:(cc + 1) * 512],
                                start=(pt == 0), stop=(pt == NPT - 1),
                            )
                        dst = osb[:, cc * 512:(cc + 1) * 512]
                        # split PSUM drains DVE/ACT so neither engine
                        # paces the pipeline
                        if cc == 0:
                            nc.vector.tensor_copy(dst, ops)
                        else:
                            nc.scalar.copy(dst, ops)
                        # out-DMAs issue from sync (idle in D); only the
                        # final tile splits across both HWDGE engines
                        eng = nc.sync
                        if ch == NCH - 1 and st == NST - 1:
                            # final tile: row halves on two queues to
                            # halve the end-of-kernel drain
                            for rh in range(2):
                                e2 = nc.sync if rh == 0 else nc.scalar
                                e2.dma_start(
                                    out[row + rh * 64:row + (rh + 1) * 64,
                                        cc * 512:(cc + 1) * 512],
                                    osb[rh * 64:(rh + 1) * 64,
                                        cc * 512:(cc + 1) * 512],
                                )
                        else:
                            eng.dma_start(
                                out[row:row + 128, cc * 512:(cc + 1) * 512],
                                osb[:, cc * 512:(cc + 1) * 512],
                            )
                    if st == NST - 1:
                        ho_tiles.pop(ch)

                items = [(ch, pt) for ch in range(NCH) for pt in range(NPT)]
                DEPTH = 4
                ex_tiles = {}
                for i in range(DEPTH):
                    ex_tiles[items[i]] = stage_a(*items[i])
                for i, (ch, pt) in enumerate(items):
                    if i + DEPTH < len(items):
                        ex_tiles[items[i + DEPTH]] = stage_a(*items[i + DEPTH])
                    # outproj (independent PE work) is queued BEFORE
                    # stage_b: the PE runs its queue in order. The spread
                    # is shifted one item late so an item never runs an
                    # outproj whose ho normalize finished only one item
                    # ago: item (ch,0) runs the two-chunks-old final
                    # s-tile instead (needs p2ho bufs=3).
                    if pt == 0:
                        if ch >= 2:
                            outproj_st(ch - 2, NST - 1)
                    elif ch >= 1:
                        outproj_st(ch - 1, pt - 1)
                    stage_b(ch, pt, ex_tiles.pop((ch, pt)))
                outproj_st(NCH - 2, NST - 1)
                for st in range(NST):
                    outproj_st(NCH - 1, st)

    nc.compile()
    return nc


def get_compiled():
    global _compiled
    with _lock:
        if _compiled is None:
            _compiled = _build()
    return _compiled


def make_in_maps(x, Wq, Wk, Wv, E_w, E_b, F_w, F_b, Wo, bo):
    """Host-side sharding: core i -> (batch i//2, head-group i%2)."""
    import ml_dtypes

    f = np.float32
    bf = ml_dtypes.bfloat16
    x = np.asarray(x, f)
    efT = np.ascontiguousarray(
        np.concatenate([np.asarray(E_w, f).T, np.asarray(F_w, f).T], axis=1)
    ).astype(bf)                                        # [S, 2*DK]
    in_maps = []
    for core in range(NCORES):
        b, g = divmod(core, 2)
        hs = slice(g * HG, (g + 1) * HG)
        wq = np.asarray(Wq, f)[hs].reshape(HD, C)
        wk = np.asarray(Wk, f)[hs].reshape(HD, C)
        wv = np.asarray(Wv, f)[hs].reshape(HD, C)
        wo = np.asarray(Wo, f)[:, g * HD:(g + 1) * HD]      # [C, 512]
        in_maps.append({
            "xs": np.ascontiguousarray(x[b]).astype(bf),    # [S, C]
            "xt": np.ascontiguousarray(x[b].T).astype(bf),  # [C, S]
            "eft": efT,
            "wqt": np.ascontiguousarray(wq.T).astype(bf),   # [C, HD]
            "wkt": np.ascontiguousarray(wk.T).astype(bf),
            "wvt": np.ascontiguousarray(wv.T).astype(bf),
            "eb": np.asarray(E_b, f),
            "fb": np.asarray(F_b, f),
            "wot": np.ascontiguousarray(wo.T).astype(bf),   # [HD, C]
            "ones": np.ones((128, 64), bf),
        })
    return in_maps


def assemble(results, bo):
    out = np.empty((B, S, C), np.float32)
    for b in range(B):
        out[b] = (
            np.asarray(results[2 * b]["out"], np.float32)
            + np.asarray(results[2 * b + 1]["out"], np.float32)
        )
    out += np.asarray(bo, np.float32)[None, None, :]
    return out


def kernel(x, Wq, Wk, Wv, E_w, E_b, F_w, F_b, Wo, bo):
    from concourse.bass_utils import run_bass_kernel_spmd

    nc = get_compiled()
    in_maps = make_in_maps(x, Wq, Wk, Wv, E_w, E_b, F_w, F_b, Wo, bo)
    res = run_bass_kernel_spmd(nc, in_maps, core_ids=list(range(NCORES)))
    return assemble(res.results, bo)
